# revision 1
# baseline (speedup 1.0000x reference)
"""MHA on 8 TRN2 cores — v2: single flat pipeline, ACT-bound design.

Sharding: 8 shards = 4 batches x 2 head-halves (per core: 1 batch, 8 heads
= 4 head-pairs). Host sums the two half-head partials per batch + bo.

Per-core dataflow (QCH=512 q-chunks, 16 k-tiles):
  - scores: row-tiled (T0/T8) concurrent MM pairs -> PSUM ring A(4 banks,
    2 kt) / B(2 banks, 1 kt); one EXP per ring slot (N=2048 / N=1024).
  - et (fp16, SBUF): DVE accumulates softmax denominators; PV col-tiled
    concurrent MM pairs accumulate xT[dh-pair, q] in 1 PSUM bank.
  - denominators: ones-matmul partition-reduce+broadcast, fast reciprocal,
    normalization fused into PSUM->SBUF evacuation.
  - projections (QT/KT/V) + out-projection interleaved into the attention
    stream as deferred tasks on a shared PSUM bank; Q/K biases fused into
    the evacuation (tensor_scalar_add), V bias via ones-matmul.
"""

import os
from collections import deque

import numpy as np

import concourse.bass as bass
from concourse import bacc
import concourse.mybir as mybir
import concourse.tile as tile
from concourse.bass_utils import run_bass_kernel_spmd

B, S, D, H, DH = 4, 2048, 1024, 16, 64
P = 128
HC = H // 2          # heads per core = 8
PAIRS = HC // 2      # 4
DT = D // P          # 8
NKT = S // P         # 16
QCH = 512
NQC = S // QCH       # 4
HDH = HC * DH        # 512

F32 = mybir.dt.float32
F16 = mybir.dt.float16
EXP = mybir.ActivationFunctionType.Exp


def _emit(nc):
    xq = nc.dram_tensor("xq", [S, D], F16, kind="ExternalInput")
    xkv = nc.dram_tensor("xkv", [S, D], F16, kind="ExternalInput")
    wq = nc.dram_tensor("wq", [D, HDH], F16, kind="ExternalInput")
    wk = nc.dram_tensor("wk", [D, HDH], F16, kind="ExternalInput")
    wv = nc.dram_tensor("wv", [D, HDH], F16, kind="ExternalInput")
    bq = nc.dram_tensor("bq", [HDH], F32, kind="ExternalInput")
    bk = nc.dram_tensor("bk", [HDH], F32, kind="ExternalInput")
    bv = nc.dram_tensor("bv", [HDH], F16, kind="ExternalInput")
    wo = nc.dram_tensor("wo", [HDH, D], F16, kind="ExternalInput")
    out = nc.dram_tensor("out", [S, D], F32, kind="ExternalOutput")

    with tile.TileContext(nc) as tc:
        with (
            tc.tile_pool(name="pers", bufs=1) as pers,
            tc.tile_pool(name="xk", bufs=1) as xk_pool,
            tc.tile_pool(name="xq", bufs=1) as xq_pool,
            tc.tile_pool(name="w", bufs=1) as w_pool,
            tc.tile_pool(name="et", bufs=7) as et_pool,
            tc.tile_pool(name="acc", bufs=4) as acc_pool,
            tc.tile_pool(name="rec", bufs=2) as rec_pool,
            tc.tile_pool(name="xts", bufs=8) as xts_pool,
            tc.tile_pool(name="osb", bufs=4) as osb_pool,
            tc.tile_pool(name="pa", bufs=1, space="PSUM") as pa_pool,
            tc.tile_pool(name="pb", bufs=1, space="PSUM") as pb_pool,
            tc.tile_pool(name="pxt", bufs=1, space="PSUM") as pxt_pool,
            tc.tile_pool(name="psh", bufs=1, space="PSUM") as psh_pool,
        ):
            # persistent SBUF
            qt_sb = [pers.tile([P, S], F16, tag=f"qt{t}", name=f"qt{t}") for t in range(PAIRS)]
            kt_sb = [pers.tile([P, S], F16, tag=f"kt{t}", name=f"kt{t}") for t in range(PAIRS)]
            v_sb = [pers.tile([P, HDH], F16, tag=f"v{st}", name=f"v{st}") for st in range(NKT)]
            wo_sb = [pers.tile([P, D], F16, tag=f"wo{t}", name=f"wo{t}") for t in range(PAIRS)]
            ones_mm = pers.tile([1, HDH], F16, tag="ones_mm")
            ones_red = pers.tile([P, 64], F16, tag="ones_red")
            bqc = pers.tile([P, PAIRS], F32, tag="bqc")
            bkc = pers.tile([P, PAIRS], F32, tag="bkc")
            bv_sb = pers.tile([1, HDH], F16, tag="bv")

            pA = pa_pool.tile([P, 2048], F32, tag="pA")      # 4 banks
            pB = pb_pool.tile([P, 1024], F32, tag="pB")      # 2 banks
            pXT = pxt_pool.tile([P, QCH], F32, tag="pXT")    # 1 bank
            pSH = psh_pool.tile([P, 512], F32, tag="pSH")    # 1 bank

            nc.vector.memset(ones_mm, 1.0)
            nc.vector.memset(ones_red, 1.0)

            # ---- input DMA (need-order; queues run ahead of compute) ----
            xkv_t = [[None] * 2 for _ in range(DT)]   # [d][half] -> [128,1024]
            xq_t = [[None] * NQC for _ in range(DT)]  # [d][c] -> [128,512]
            for d in range(DT):
                xkv_t[d][0] = xk_pool.tile([P, 1024], F16, tag=f"xkv{d}_0", name=f"xkv{d}_0")
                nc.sync.dma_start_transpose(
                    out=xkv_t[d][0], in_=xkv[0:1024, d * P : (d + 1) * P]
                )
            wk_t = [w_pool.tile([P, HDH], F16, tag=f"wk{d}", name=f"wk{d}") for d in range(DT)]
            for d in range(DT):
                nc.sync.dma_start(out=wk_t[d], in_=wk[d * P : (d + 1) * P, :])
            for d in range(DT):
                xq_t[d][0] = xq_pool.tile([P, QCH], F16, tag=f"xq{d}_0", name=f"xq{d}_0")
                nc.sync.dma_start_transpose(out=xq_t[d][0], in_=xq[0:QCH, d * P : (d + 1) * P])
            wq_t = [w_pool.tile([P, HDH], F16, tag=f"wq{d}", name=f"wq{d}") for d in range(DT)]
            for d in range(DT):
                nc.sync.dma_start(out=wq_t[d], in_=wq[d * P : (d + 1) * P, :])
            for d in range(DT):
                xkv_t[d][1] = xk_pool.tile([P, 1024], F16, tag=f"xkv{d}_1", name=f"xkv{d}_1")
                nc.sync.dma_start_transpose(
                    out=xkv_t[d][1], in_=xkv[1024:2048, d * P : (d + 1) * P]
                )
            for p_ in range(PAIRS):
                nc.sync.dma_start(out=bqc[:, p_ : p_ + 1], in_=bq[p_ * P : (p_ + 1) * P, None])
                nc.sync.dma_start(out=bkc[:, p_ : p_ + 1], in_=bk[p_ * P : (p_ + 1) * P, None])
            wv_t = [w_pool.tile([P, HDH], F16, tag=f"wv{d}", name=f"wv{d}") for d in range(DT)]
            for d in range(DT):
                nc.sync.dma_start(out=wv_t[d], in_=wv[d * P : (d + 1) * P, :])
            nc.sync.dma_start(out=bv_sb, in_=bv[None, :])
            for c in range(1, NQC):
                for d in range(DT):
                    xq_t[d][c] = xq_pool.tile([P, QCH], F16, tag=f"xq{d}_{c}", name=f"xq{d}_{c}")
                    nc.sync.dma_start_transpose(
                        out=xq_t[d][c], in_=xq[c * QCH : (c + 1) * QCH, d * P : (d + 1) * P]
                    )
            for t in range(PAIRS):
                nc.sync.dma_start(out=wo_sb[t], in_=wo[t * P : (t + 1) * P, :])

            # ---- projection task helpers (psum target rotates in prologue) ----
            def kt_chunk(pair, c, ps):
                # KT[pair][:, c*512:+512] = Wk[:,pair]^T @ xkvT + bk
                for d in range(DT):
                    nc.tensor.matmul(
                        ps,
                        lhsT=wk_t[d][:, pair * P : (pair + 1) * P],
                        rhs=xkv_t[d][c // 2][:, (c % 2) * 512 : (c % 2) * 512 + 512],
                        start=(d == 0),
                        stop=(d == DT - 1),
                    )
                nc.vector.tensor_scalar_add(
                    out=kt_sb[pair][:, c * 512 : (c + 1) * 512],
                    in0=ps,
                    scalar1=bkc[:, pair : pair + 1],
                )

            def qt_chunk(pair, c, ps):
                for d in range(DT):
                    nc.tensor.matmul(
                        ps,
                        lhsT=wq_t[d][:, pair * P : (pair + 1) * P],
                        rhs=xq_t[d][c],
                        start=(d == 0),
                        stop=(d == DT - 1),
                    )
                nc.vector.tensor_scalar_add(
                    out=qt_sb[pair][:, c * 512 : (c + 1) * 512],
                    in0=ps,
                    scalar1=bqc[:, pair : pair + 1],
                )

            def v_chunk(st, ps):
                for d in range(DT):
                    nc.tensor.matmul(
                        ps,
                        lhsT=xkv_t[d][st // 8][:, (st % 8) * P : (st % 8) * P + P],
                        rhs=wv_t[d],
                        start=(d == 0),
                        stop=False,
                    )
                nc.tensor.matmul(
                    ps, lhsT=ones_mm[:, :P], rhs=bv_sb, start=False, stop=True
                )
                nc.vector.tensor_copy(out=v_sb[st], in_=ps)

            # ---- prologue: KT[p0], QT[p0,c0], V[0..7] on rotating psum ----
            rot = [pSH, pB[:, 0:512], pA[:, 0:512], pA[:, 512:1024],
                   pB[:, 512:1024], pA[:, 1024:1536], pA[:, 1536:2048]]
            ri = 0

            def nxt():
                nonlocal ri
                t_ = rot[ri % len(rot)]
                ri += 1
                return t_

            for c in range(2):
                kt_chunk(0, c, nxt())
            qt_chunk(0, 0, nxt())
            for st in range(8):
                v_chunk(st, nxt())

            # ---- deferred work (1 task per kt slot) ----
            work = deque()
            work.append(("kt", 0, 2))
            work.append(("kt", 0, 3))
            for st in range(8, NKT):
                work.append(("v", st))
            for c in range(4):
                work.append(("kt", 1, c))
            work.append(("qt", 1, 0))

            def push_unit_work(qc, pair):
                # schedule prerequisites for units after (qc, pair)
                if qc == 0 and pair in (0, 1):
                    np_ = pair + 2
                    for c in range(4):
                        work.append(("kt", np_, c))
                    work.append(("qt", np_, 0))
                # QT for the unit 3 ahead (early qc0 pairs already scheduled)
                idx = qc * PAIRS + pair + 3
                if PAIRS <= idx < NQC * PAIRS:
                    work.append(("qt", idx % PAIRS, idx // PAIRS))

            def emit_task():
                if not work:
                    return
                t_ = work.popleft()
                if t_[0] == "v":
                    v_chunk(t_[1], pSH)
                elif t_[0] == "kt":
                    kt_chunk(t_[1], t_[2], pSH)
                elif t_[0] == "qt":
                    qt_chunk(t_[1], t_[2], pSH)
                else:
                    op_group(t_[1], t_[2], t_[3], pSH)

            # ---- attention stream ----
            units = [(qc, pair) for qc in range(NQC) for pair in range(PAIRS)]
            ustate = {}
            for u in units:
                ustate[u] = {"done": 0, "acc0": None, "acc1": None}

            pending = []  # (unit, kt, et_tile, base)
            pvq = deque()  # lagged PV work: (unit, kt, et_tile, base)
            ktg = 0

            def drain(et):
                for (u, kt, ett, base) in pending:
                    st_ = ustate[u]
                    # denominator partial sums
                    if kt == 0:
                        st_["acc0"] = acc_pool.tile([P, QCH], F16, tag="acc", name="acc")
                        st_["acc1"] = acc_pool.tile([P, QCH], F16, tag="acc", name="acc")
                        nc.vector.tensor_copy(out=st_["acc0"], in_=ett[:, base : base + 512])
                        nc.vector.tensor_copy(out=st_["acc1"], in_=ett[:, base + 512 : base + 1024])
                    else:
                        nc.vector.tensor_add(out=st_["acc0"], in0=st_["acc0"], in1=ett[:, base : base + 512])
                        nc.vector.tensor_add(out=st_["acc1"], in0=st_["acc1"], in1=ett[:, base + 512 : base + 1024])
                    pvq.append((u, kt, ett, base))
                pending.clear()

            def emit_pv(lag):
                # emit PV pairs whose exp is at least `lag` entries back, so the
                # PE stream never blocks on the in-flight EXP
                while len(pvq) > lag:
                    u, kt, ett, base = pvq.popleft()
                    qc, pair = u
                    h0, h1 = 2 * pair, 2 * pair + 1
                    nc.tensor.matmul(
                        pXT[0:64, :],
                        lhsT=v_sb[kt][:, h0 * DH : (h0 + 1) * DH],
                        rhs=ett[:, base : base + 512],
                        start=(kt == 0),
                        stop=(kt == NKT - 1),
                        tile_position=(0, 0),
                        skip_group_check=True,
                    )
                    nc.tensor.matmul(
                        pXT[64:128, :],
                        lhsT=v_sb[kt][:, h1 * DH : (h1 + 1) * DH],
                        rhs=ett[:, base + 512 : base + 1024],
                        start=(kt == 0),
                        stop=(kt == NKT - 1),
                        tile_position=(0, 64),
                        skip_group_check=True,
                    )
                    st_ = ustate[u]
                    st_["done"] += 1
                    if st_["done"] == NKT:
                        finish_unit(u)

            xts_map = {}

            def finish_unit(u):
                qc, pair = u
                st_ = ustate[u]
                # denominators: partition reduce + broadcast (col-tiled pair)
                nc.tensor.matmul(
                    pSH[0:64, :], lhsT=ones_red, rhs=st_["acc0"],
                    start=True, stop=True, tile_position=(0, 0), skip_group_check=True,
                )
                nc.tensor.matmul(
                    pSH[64:128, :], lhsT=ones_red, rhs=st_["acc1"],
                    start=True, stop=True, tile_position=(0, 64), skip_group_check=True,
                )
                rec = rec_pool.tile([P, QCH], F32, tag="rec")
                nc.vector.reciprocal_approx_fast(out=rec, in_=pSH)
                xt_sb = xts_pool.tile([P, QCH], F16, tag="xts")
                nc.vector.tensor_mul(out=xt_sb, in0=pXT, in1=rec)
                xts_map[u] = xt_sb
                if pair == PAIRS - 1:
                    out_proj(qc)

            def op_group(qc, qt_, dc, ps):
                for pr in range(PAIRS):
                    nc.tensor.matmul(
                        ps,
                        lhsT=xts_map[(qc, pr)][:, qt_ * P : (qt_ + 1) * P],
                        rhs=wo_sb[pr][:, dc * 512 : (dc + 1) * 512],
                        start=(pr == 0),
                        stop=(pr == PAIRS - 1),
                    )
                osb = osb_pool.tile([P, 512], F32, tag="osb")
                nc.vector.tensor_copy(out=osb, in_=ps)
                q0 = qc * QCH + qt_ * P
                nc.gpsimd.dma_start(
                    out=out[q0 : q0 + P, dc * 512 : (dc + 1) * 512], in_=osb
                )

            def out_proj(qc):
                for qt_ in range(QCH // P):
                    for dc in range(D // 512):
                        if qc < NQC - 1:
                            work.append(("op", qc, qt_, dc))
                        else:
                            op_group(qc, qt_, dc, nxt())

            for (qc, pair) in units:
                push_unit_work(qc, pair)
                for kt in range(NKT):
                    slot = ktg % 3
                    ktg += 1
                    if slot == 0:
                        dst, base, cur = pA, 0, None
                    elif slot == 1:
                        dst, base, cur = pA, 1024, pA
                    else:
                        dst, base, cur = pB, 0, pB
                    nc.tensor.matmul(
                        dst[:, base : base + 512],
                        lhsT=kt_sb[pair][0:64, kt * P : (kt + 1) * P],
                        rhs=qt_sb[pair][0:64, qc * QCH : (qc + 1) * QCH],
                        start=True, stop=True, tile_position=(0, 0),
                    )
                    nc.tensor.matmul(
                        dst[:, base + 512 : base + 1024],
                        lhsT=kt_sb[pair][64:128, kt * P : (kt + 1) * P],
                        rhs=qt_sb[pair][64:128, qc * QCH : (qc + 1) * QCH],
                        start=True, stop=True, tile_position=(64, 0),
                    )
                    pending.append(((qc, pair), kt, None, base if slot != 1 else 1024))
                    emit_task()
                    if cur is not None:
                        n = 2048 if cur is pA else 1024
                        et = et_pool.tile([P, 2048], F16, tag="et")
                        nc.scalar.activation(out=et[:, 0:n], in_=cur, func=EXP, scale=0.125)
                        fixed = [(u, kt_, et, b_) for (u, kt_, _, b_) in pending]
                        pending.clear()
                        pending.extend(fixed)
                        drain(et)
                    emit_pv(3)

            # flush: rotation may end mid-A-group (256 % 3 == 1 leftover kt)
            if pending:
                et = et_pool.tile([P, 2048], F16, tag="et", name="et_flush")
                nc.scalar.activation(out=et[:, 0:1024], in_=pA[:, 0:1024], func=EXP, scale=0.125)
                fixed = [(u, kt_, et, b_) for (u, kt_, _, b_) in pending]
                pending.clear()
                pending.extend(fixed)
                drain(et)
            emit_pv(0)
            while work:
                emit_task()

    return nc


_NC_CACHE = None
LAST_RESULTS = None


def _get_nc():
    global _NC_CACHE
    if _NC_CACHE is None:
        nc = bacc.Bacc(None, target_bir_lowering=False)
        _emit(nc)
        nc.compile()
        _NC_CACHE = nc
    return _NC_CACHE


def kernel(**inputs):
    global LAST_RESULTS
    inputs_q = np.ascontiguousarray(inputs["inputs_q"], np.float16)
    inputs_kv = np.ascontiguousarray(inputs["inputs_kv"], np.float16)
    Wq = np.asarray(inputs["Wq"], np.float16)
    Wk = np.asarray(inputs["Wk"], np.float16)
    Wv = np.asarray(inputs["Wv"], np.float16)
    bq = np.asarray(inputs["bq"], np.float32)
    bk = np.asarray(inputs["bk"], np.float32)
    bv = np.asarray(inputs["bv"], np.float16)
    Wo = np.asarray(inputs["Wo"], np.float16)
    bo = np.asarray(inputs["bo"], np.float32)

    nc = _get_nc()

    in_maps = []
    for core in range(8):
        b, g = core // 2, core % 2
        hsl = slice(g * HC, (g + 1) * HC)
        in_maps.append(
            {
                "xq": inputs_q[b],
                "xkv": inputs_kv[b],
                "wq": np.ascontiguousarray(Wq[:, hsl, :].reshape(D, HDH)),
                "wk": np.ascontiguousarray(Wk[:, hsl, :].reshape(D, HDH)),
                "wv": np.ascontiguousarray(Wv[:, hsl, :].reshape(D, HDH)),
                "bq": np.ascontiguousarray(bq[hsl].reshape(HDH)),
                "bk": np.ascontiguousarray(bk[hsl].reshape(HDH)),
                "bv": np.ascontiguousarray(bv[hsl].reshape(HDH)),
                "wo": np.ascontiguousarray(Wo[hsl].reshape(HDH, D)),
            }
        )

    res = run_bass_kernel_spmd(
        nc,
        in_maps,
        core_ids=list(range(8)),
        trace=bool(int(os.environ.get("KERNEL_TRACE", "0"))),
    )
    LAST_RESULTS = res

    out = np.empty((B, S, D), np.float32)
    for b in range(B):
        out[b] = res.results[2 * b]["out"] + res.results[2 * b + 1]["out"] + bo
    return out

